# revision 6
# baseline (speedup 1.0000x reference)
"""Trainium2 Bass kernel v2: batched affine bilinear sampling via dma_gather.

Full inputs: images [32, 512, 512, 3] f32, theta [32, 2, 3] f32.
Data parallel over batch: 8 NeuronCores x 4 images; one SPMD launch per image
(4 output blocks of 128 rows per launch).

Device algorithm per launch:
  1. Stage an fp16 "quad-slot" image in DRAM: slot(y*512+x) = 32B =
     [img[y,x], img[y,x+1], img[y+1,x], img[y+1,x+1]] as 12 fp16 + pad.
     8 slots = one 256B gather element; element index = (y*512+x)>>3 <= 32767
     fits int16.
  2. Per block: compute exact sample coords / lerp weights (baseline DVE
     recipe), plus element index idx = y0*64 + (x0>>3) and slot-in-element
     o = x0 & 7.
  3. Shuffle idx into dma_gather's wrapped index layout
     (table[q, j*8+u] = idx[i=16u+q, j]) via int16 DMA-transpose + a
     strided DRAM round trip, broadcast to all 8 gpsimd index stripes.
  4. 64 dma_gathers per block (1024 idxs each - ucode cap), prepare_only +
     trigger, 8 rotating completion sems; each element lands 8 candidate
     slots per pixel on the pixel's output-row partition.
  5. Mux the right slot with a one-hot multiply + strided tensor_reduce,
     then bilinear-blend with f32 weights and store the block.
"""

import sys
from contextlib import ExitStack

for _p in ("/opt/trn_rl_repo",):
    if _p not in sys.path:
        sys.path.append(_p)

import numpy as np

import concourse.bacc as bacc
import concourse.bass as bass
import concourse.tile as tile
from concourse import library_config, mybir
from concourse.bass_utils import run_bass_kernel_spmd

F32 = mybir.dt.float32
F16 = mybir.dt.float16
I16 = mybir.dt.int16
OP = mybir.AluOpType
ACTF = mybir.ActivationFunctionType
AX = mybir.AxisListType

H = W = 512
P = 128
NBLK = H // P
MAGIC = float(2 ** 23)
N_CORES = 8
BPL = 4            # blocks per launch
NI = 1024          # idxs per dma_gather (ucode cap)
GPB = W // 8       # gathers per block (8 cols each) = 64
NGRP = 8           # mux groups per block (8 gathers / 64 cols each)
NE16 = H * W // 8  # 32768 gather elements


def _floor_exact(nc, pool, v, name):
    """Exact floor of f32 tensor v (|v| < 2^22) -> new tile, via magic round
    + compare fixup. Returns r = floor(v)."""
    r = pool.tile([P, W], F32, name=f"fl_{name}")
    nc.scalar.activation(out=r, in_=v, func=ACTF.Copy, bias=MAGIC)
    nc.scalar.activation(out=r, in_=r, func=ACTF.Copy, bias=-MAGIC)
    g = pool.tile([P, W], F32, name="flg")
    nc.vector.tensor_tensor(out=g, in0=r, in1=v, op=OP.is_gt)
    nc.vector.tensor_sub(r, r, g)
    return r


def _body(ctx: ExitStack, tc: "tile.TileContext", imgs: bass.AP,
          theta: bass.AP, bb: bass.AP, gxr: bass.AP, pr: bass.AP,
          prs: bass.AP, out: bass.AP):
    nc = tc.nc

    imgQ = nc.dram_tensor("imgQ16", [NE16, 128], F16, kind="Internal").ap()
    tabD = [nc.dram_tensor(f"tabD{i}", [16, W * 8], I16, kind="Internal").ap()
            for i in range(2)]

    stg_sem = nc.alloc_semaphore(name="stg_sem")
    idx_sem = nc.alloc_semaphore(name="idx_sem")
    tld_sem = nc.alloc_semaphore(name="tld_sem")
    gsems = [nc.alloc_semaphore(name=f"gs{i}") for i in range(16)]

    nc.gpsimd.load_library(library_config.mlp)
    nireg = nc.gpsimd.to_reg(NI)

    const_pool = ctx.enter_context(tc.tile_pool(name="const", bufs=1))
    stage_pool = ctx.enter_context(tc.tile_pool(name="stage", bufs=2))
    pairs_pool = ctx.enter_context(tc.tile_pool(name="pairs", bufs=2))
    coord_pool = ctx.enter_context(tc.tile_pool(name="coord", bufs=1))
    late_pool = ctx.enter_context(tc.tile_pool(name="late", bufs=2))
    tiny_pool = ctx.enter_context(tc.tile_pool(name="tiny", bufs=2))
    tt_pool = ctx.enter_context(tc.tile_pool(name="tt", bufs=2))
    idx_pool = ctx.enter_context(tc.tile_pool(name="idxp", bufs=2))
    pay_pool = ctx.enter_context(tc.tile_pool(name="pay", bufs=2))
    prod_pool = ctx.enter_context(tc.tile_pool(name="prod", bufs=1))
    quad_pool = ctx.enter_context(tc.tile_pool(name="quadm", bufs=1))
    outb_pool = ctx.enter_context(tc.tile_pool(name="outb", bufs=2))

    # --- constants ---
    th = const_pool.tile([P, 6], F32)
    nc.sync.dma_start(out=th, in_=theta.unsqueeze(0).to_broadcast([P, 6]))
    bbs = const_pool.tile([P, BPL], F32)
    nc.sync.dma_start(out=bbs, in_=bb.unsqueeze(0).to_broadcast([P, BPL]))
    gx = const_pool.tile([P, W], F32)   # -1 + j*2/511 ramp on every partition
    nc.sync.dma_start(out=gx, in_=gxr.unsqueeze(0).to_broadcast([P, W]))
    pcol = const_pool.tile([P, 1], F32)  # partition index 0..127
    nc.sync.dma_start(out=pcol, in_=pr.unsqueeze(1))
    pcols = const_pool.tile([P, 1], F32)  # sigma(p) = (p%8)*16 + p//8
    nc.sync.dma_start(out=pcols, in_=prs.unsqueeze(1))

    a_ = th[:, 0:1]; b_ = th[:, 1:2]; c_ = th[:, 2:3]
    d_ = th[:, 3:4]; e_ = th[:, 4:5]; f_ = th[:, 5:6]

    # --- staging: build fp16 quad-slot image in DRAM ---
    imgs_flat = imgs.rearrange("k h w c -> k (h w c)")
    for blk in range(NBLK):
        r0 = blk * P
        loadAB = stage_pool.tile([P, 2, (W + 1) * 3], F32)
        src = bass.AP(
            tensor=imgs_flat.tensor,
            offset=imgs_flat.offset + r0 * W * 3,
            ap=[[W * 3, P], [W * 3, 2], [1, (W + 1) * 3]],
        )
        nc.scalar.dma_start(out=loadAB, in_=src)
        pairs = pairs_pool.tile([P, W, 16], F16)
        flat0 = loadAB[:, 0, :]
        flat1 = loadAB[:, 1, :]
        win0 = bass.AP(tensor=flat0.tensor, offset=flat0.offset,
                       ap=[flat0.ap[0], [3, W], [1, 6]])
        win1 = bass.AP(tensor=flat1.tensor, offset=flat1.offset,
                       ap=[flat1.ap[0], [3, W], [1, 6]])
        win0p = bass.AP(tensor=flat0.tensor, offset=flat0.offset,
                        ap=[flat0.ap[0], [3, W], [1, 4]])
        cv = nc.vector.tensor_copy(out=pairs[:, :, 0:6], in_=win0)
        if blk >= 1:
            cv._wait_ge(stg_sem, 16 * blk)
        cva = nc.scalar.activation(out=pairs[:, :, 6:12], in_=win1,
                                   func=ACTF.Copy)
        cvp = nc.vector.tensor_copy(out=pairs[:, :, 12:16], in_=win0p)
        if blk >= 1:
            cva._wait_ge(stg_sem, 16 * blk)
            cvp._wait_ge(stg_sem, 16 * blk)
        st = nc.gpsimd.dma_start(
            out=bass.AP(tensor=imgQ.tensor, offset=imgQ.offset + r0 * W * 16,
                        ap=[[W * 16, P], [1, W * 16]]),
            in_=pairs.rearrange("p w c -> p (w c)"))
        st.then_inc(stg_sem, 16)

    # --- per-output-block pipeline ---
    A256 = tiny_pool.tile([P, 1], F32, name="A256")
    nc.vector.tensor_scalar_mul(A256, a_, 256.0)
    D256 = tiny_pool.tile([P, 1], F32, name="D256")
    nc.vector.tensor_scalar_mul(D256, d_, 256.0)
    c1x = tiny_pool.tile([P, 1], F32, name="c1x")
    nc.vector.tensor_scalar(out=c1x, in0=c_, scalar1=1.0, scalar2=256.0,
                            op0=OP.add, op1=OP.mult)
    c1y = tiny_pool.tile([P, 1], F32, name="c1y")
    nc.vector.tensor_scalar(out=c1y, in0=f_, scalar1=1.0, scalar2=256.0,
                            op0=OP.add, op1=OP.mult)
    xa = tiny_pool.tile([P, W], F32, name="xa")
    nc.vector.tensor_scalar(out=xa, in0=gx, scalar1=A256, scalar2=None,
                            op0=OP.mult)
    ya = tiny_pool.tile([P, W], F32, name="ya")
    nc.vector.tensor_scalar(out=ya, in0=gx, scalar1=D256, scalar2=None,
                            op0=OP.mult)

    imgQ_ap = bass.AP(tensor=imgQ.tensor, offset=imgQ.offset,
                      ap=[[128, NE16], [1, 128]])

    for q in range(BPL):
        gyb = tiny_pool.tile([P, 1], F32, name="gyb")
        nc.vector.tensor_scalar(out=gyb, in0=pcol, scalar1=512.0 / 511.0,
                                scalar2=bbs[:, q:q + 1], op0=OP.mult,
                                op1=OP.add)
        sx = tiny_pool.tile([P, 1], F32, name="sx")
        nc.vector.tensor_scalar(out=sx, in0=gyb, scalar1=b_, scalar2=c1x,
                                op0=OP.mult, op1=OP.add)
        sy = tiny_pool.tile([P, 1], F32, name="sy")
        nc.vector.tensor_scalar(out=sy, in0=gyb, scalar1=e_, scalar2=c1y,
                                op0=OP.mult, op1=OP.add)

        def coord_side(arow, scol, tag):
            v = late_pool.tile([P, W], F32, name=f"v{tag}")
            nc.vector.tensor_scalar(out=v, in0=arow, scalar1=scol,
                                    scalar2=None, op0=OP.add)
            r = _floor_exact(nc, coord_pool, v, tag)
            nc.vector.tensor_scalar(out=r, in0=r, scalar1=0.0, scalar2=511.0,
                                    op0=OP.max, op1=OP.min)
            p1 = late_pool.tile([P, W], F32, name=f"p1{tag}")
            nc.vector.tensor_scalar(out=p1, in0=r, scalar1=1.0, scalar2=511.0,
                                    op0=OP.add, op1=OP.min)
            nc.vector.tensor_scalar(out=v, in0=v, scalar1=0.0, scalar2=511.0,
                                    op0=OP.max, op1=OP.min)
            nc.vector.tensor_sub(p1, p1, v)   # u0 = x1 - xc
            nc.vector.tensor_sub(v, v, r)     # u1 = xc - x0
            return p1, v, r

        u0, u1, x0f = coord_side(xa, sx, "x")
        v0, v1, y0f = coord_side(ya, sy, "y")

        # o = x0f & 7 (natural row order, for the mux one-hot)
        t8 = coord_pool.tile([P, W], F32, name="t8")
        nc.vector.tensor_scalar(out=t8, in0=x0f, scalar1=0.125,
                                scalar2=MAGIC, op0=OP.mult, op1=OP.add)
        nc.scalar.activation(out=t8, in_=t8, func=ACTF.Copy, bias=-MAGIC)
        fx = coord_pool.tile([P, W], F32, name="fx")
        nc.vector.scalar_tensor_tensor(out=fx, in0=t8, scalar=8.0,
                                       in1=x0f, op0=OP.mult, op1=OP.is_gt)
        nc.vector.tensor_sub(t8, t8, fx)      # t8 = xq
        o = coord_pool.tile([P, W], F32, name="o")
        nc.vector.scalar_tensor_tensor(out=o, in0=t8, scalar=-8.0,
                                       in1=x0f, op0=OP.mult, op1=OP.add)

        # sigma-ordered idx pass: partition p computes row sigma(p), so the
        # transposed idx tile is scatter-contiguous while the gather payload
        # still lands row-natural.
        gybs = tiny_pool.tile([P, 1], F32, name="gybs")
        nc.vector.tensor_scalar(out=gybs, in0=pcols, scalar1=512.0 / 511.0,
                                scalar2=bbs[:, q:q + 1], op0=OP.mult,
                                op1=OP.add)
        sxs = tiny_pool.tile([P, 1], F32, name="sxs")
        nc.vector.tensor_scalar(out=sxs, in0=gybs, scalar1=b_, scalar2=c1x,
                                op0=OP.mult, op1=OP.add)
        sys_ = tiny_pool.tile([P, 1], F32, name="sys")
        nc.vector.tensor_scalar(out=sys_, in0=gybs, scalar1=e_, scalar2=c1y,
                                op0=OP.mult, op1=OP.add)

        def floor_clamp_s(arow, scol, tag):
            v = coord_pool.tile([P, W], F32, name=f"vs{tag}")
            nc.vector.tensor_scalar(out=v, in0=arow, scalar1=scol,
                                    scalar2=None, op0=OP.add)
            r = _floor_exact(nc, coord_pool, v, f"s{tag}")
            nc.vector.tensor_scalar(out=r, in0=r, scalar1=0.0, scalar2=511.0,
                                    op0=OP.max, op1=OP.min)
            return r

        x0s = floor_clamp_s(xa, sxs, "x")
        y0s = floor_clamp_s(ya, sys_, "y")
        t8s = coord_pool.tile([P, W], F32, name="t8s")
        nc.vector.tensor_scalar(out=t8s, in0=x0s, scalar1=0.125,
                                scalar2=MAGIC, op0=OP.mult, op1=OP.add)
        nc.scalar.activation(out=t8s, in_=t8s, func=ACTF.Copy, bias=-MAGIC)
        fxs = coord_pool.tile([P, W], F32, name="fxs")
        nc.vector.scalar_tensor_tensor(out=fxs, in0=t8s, scalar=8.0,
                                       in1=x0s, op0=OP.mult, op1=OP.is_gt)
        nc.vector.tensor_sub(t8s, t8s, fxs)   # xq (sigma order)
        idxf = coord_pool.tile([P, W], F32, name="idxf")
        nc.vector.scalar_tensor_tensor(out=idxf, in0=y0s, scalar=64.0,
                                       in1=t8s, op0=OP.mult, op1=OP.add)
        idx16 = coord_pool.tile([P, W], I16, name="idx16")
        nc.vector.tensor_copy(out=idx16, in_=idxf)

        # one-hot of o: oh[p, j, s] = (o == s), fp16
        oh = late_pool.tile([P, W, 8], F16, name="oh")
        for s in range(8):
            nc.vector.tensor_scalar(out=oh[:, :, s], in0=o,
                                    scalar1=float(s), scalar2=None,
                                    op0=OP.is_equal)

        # --- idx shuffle into wrapped layout via transpose + DRAM trip ---
        tD = tabD[q % 2]
        tT = tt_pool.tile([P, W], I16, name="tT")
        for c in range(W // P):
            tp = nc.sync.dma_start_transpose(
                out=tT[:, c * P:(c + 1) * P],
                in_=idx16[:, c * P:(c + 1) * P])
            if q >= 2 and c == 0:
                tp._wait_ge(idx_sem, 64 * (q - 1))
        for c in range(W // P):
            sc = nc.gpsimd.dma_start(
                out=bass.AP(tensor=tD.tensor, offset=tD.offset + c * P * 8,
                            ap=[[8, P], [W * 8, 16], [1, 8]]),
                in_=tT[:, c * P:(c + 1) * P])
            sc.then_inc(idx_sem, 16)
            if q >= 2:
                sc._wait_ge(tld_sem, 16 * (q - 1))
        idx_sb = idx_pool.tile([P, W * 8], I16, name="idxsb")
        ld = nc.gpsimd.dma_start(
            out=idx_sb,
            in_=bass.AP(tensor=tD.tensor, offset=tD.offset,
                        ap=[[0, 8], [W * 8, 16], [1, W * 8]]))
        ld._wait_ge(idx_sem, 64 * (q + 1))
        ld.then_inc(tld_sem, 16)

        # --- gathers + mux + blend per 64-col group ---
        outblk = outb_pool.tile([P, W, 3], F32, name="outblk")
        for grp in range(NGRP):
            gg = q * NGRP + grp  # global group id
            pay = pay_pool.tile([P, 64, 128], F16, name="pay")
            for l in range(8):
                g = grp * 8 + l
                prep = nc.gpsimd.dma_gather(
                    out_ap=pay[:, l * 8:(l + 1) * 8, :],
                    in_ap=imgQ_ap,
                    idxs_ap=idx_sb[:, g * 64:(g + 1) * 64],
                    num_idxs=NI,
                    num_idxs_reg=nireg,
                    elem_size=128,
                    prepare_only=True,
                    sem=gsems[(gg * 8 + l) % 16],
                )
                prep._wait_ge(tld_sem, 16 * (q + 1))
                trig = nc.gpsimd.trigger_dma(count=None)
                ggl = gg * 8 + l
                lane = ggl % 16
                if ggl < 8:
                    trig._wait_ge(stg_sem, 64)
                elif ggl >= 16:
                    trig._wait_ge(gsems[lane], 16 * (ggl // 16))
            sl = slice(grp * 64, (grp + 1) * 64)
            prod = prod_pool.tile([P, 64, 12, 8], F16, name="prod")
            for l in range(8):
                ohb = bass.AP(tensor=oh.tensor,
                              offset=oh.offset + (grp * 64 + l * 8) * 8,
                              ap=[oh.ap[0], [8, 8], [0, 12], [1, 8]])
                payb = bass.AP(tensor=pay.tensor,
                               offset=pay.offset + l * 8 * 128,
                               ap=[pay.ap[0], [128, 8], [1, 12], [16, 8]])
                prodb = bass.AP(tensor=prod.tensor,
                                offset=prod.offset + l * 8 * 96,
                                ap=[prod.ap[0], [96, 8], [8, 12], [1, 8]])
                pm = nc.vector.tensor_mul(prodb, ohb, payb)
                ggl = gg * 8 + l
                pm._wait_ge(gsems[ggl % 16], 16 * (ggl // 16 + 1))
            quadm = quad_pool.tile([P, 64, 12], F32, name="quadm")
            nc.vector.tensor_reduce(out=quadm, in_=prod, axis=AX.X,
                                    op=OP.add)
            # blend: out = v0*(u0*P00 + u1*P01) + v1*(u0*P10 + u1*P11)
            def wb(wt):
                return bass.AP(tensor=wt.tensor,
                               offset=wt.offset + grp * 64,
                               ap=[wt.ap[0], [1, 64], [0, 3]])
            acc0 = quad_pool.tile([P, 64, 3], F32, name="acc0")
            tmp = quad_pool.tile([P, 64, 3], F32, name="tmpb")
            nc.vector.tensor_mul(acc0, quadm[:, :, 0:3], wb(u0))
            nc.vector.tensor_mul(tmp, quadm[:, :, 3:6], wb(u1))
            nc.vector.tensor_add(acc0, acc0, tmp)
            acc1 = quad_pool.tile([P, 64, 3], F32, name="acc1")
            nc.vector.tensor_mul(acc1, quadm[:, :, 6:9], wb(u0))
            nc.vector.tensor_mul(tmp, quadm[:, :, 9:12], wb(u1))
            nc.vector.tensor_add(acc1, acc1, tmp)
            nc.vector.tensor_mul(acc0, acc0, wb(v0))
            nc.vector.tensor_mul(acc1, acc1, wb(v1))
            nc.vector.tensor_add(outblk[:, sl, :], acc0, acc1)
        nc.sync.dma_start(out=out[q], in_=outblk)


def build_kernel2(num_devices: int = N_CORES):
    nc = bacc.Bacc("TRN2", target_bir_lowering=False, debug=False,
                   num_devices=num_devices)
    imgs = nc.dram_tensor("imgs", [1, H + 2, W, 3], F32, kind="ExternalInput")
    theta = nc.dram_tensor("theta", [6], F32, kind="ExternalInput")
    bb = nc.dram_tensor("bb", [BPL], F32, kind="ExternalInput")
    gxr = nc.dram_tensor("gxr", [W], F32, kind="ExternalInput")
    pr = nc.dram_tensor("pr", [P], F32, kind="ExternalInput")
    prs = nc.dram_tensor("prs", [P], F32, kind="ExternalInput")
    out = nc.dram_tensor("out", [BPL, P, W, 3], F32, kind="ExternalOutput")
    with tile.TileContext(nc) as tc:
        with ExitStack() as ctx:
            _body(ctx, tc, imgs.ap(), theta.ap(), bb.ap(), gxr.ap(), pr.ap(),
                  prs.ap(), out.ap())
    nc.compile()
    return nc


_NC_CACHE = {}


def run_kernel_spmd(images: np.ndarray, theta: np.ndarray, trace: bool = False):
    B = images.shape[0]
    per = B // N_CORES
    if "k2" not in _NC_CACHE:
        _NC_CACHE["k2"] = build_kernel2(N_CORES)
    nc = _NC_CACHE["k2"]

    out = np.zeros((B, H, W, 3), np.float32)
    slabs = []
    for c in range(N_CORES):
        s = np.zeros((per, H + 2, W, 3), np.float32)
        s[:, :H] = images[c * per:(c + 1) * per]
        slabs.append(s)

    gxr = (np.arange(W, dtype=np.float32) * (2.0 / 511.0) - 1.0).astype(
        np.float32)
    prv = np.arange(P, dtype=np.float32)
    pi = np.arange(P)
    prsv = ((pi % 8) * 16 + pi // 8).astype(np.float32)
    bbv = np.array([128.0 * q * (512.0 / 511.0) - 256.0 for q in range(BPL)],
                   np.float32)

    last_res = None
    for k in range(per):
        in_maps = []
        for c in range(N_CORES):
            in_maps.append({
                "imgs": slabs[c][k:k + 1],
                "theta": np.ascontiguousarray(
                    theta[c * per + k].reshape(-1)).astype(np.float32),
                "bb": bbv,
                "gxr": gxr,
                "pr": prv,
                "prs": prsv,
            })
        res = run_bass_kernel_spmd(nc, in_maps, core_ids=list(range(N_CORES)),
                                   trace=trace)
        last_res = res
        for c in range(N_CORES):
            out[c * per + k] = res.results[c]["out"].reshape(H, W, 3)
    return out, last_res


def kernel(images: np.ndarray, theta: np.ndarray) -> np.ndarray:
    images = np.ascontiguousarray(np.asarray(images), dtype=np.float32)
    theta = np.asarray(theta).astype(np.float32)
    out, _ = run_kernel_spmd(images, theta, trace=False)
    return out
